# revision 5
# baseline (speedup 1.0000x reference)
"""Trainium2 Bass kernel for CarefulSparseResNet3D.

Sharding: 8 cores = batch(4) x D-half(2). Each core computes its output
D-half from an overlapping input slice (compute-with-halo, no collectives).

Device layout: activations are H-fold partitioned: partition p = ch + 64*fold,
fold = H half (h 0..15 / 16..31). All convs run as 4-way PE-tiled (64x64)
matmul accumulations over "quads" (2 consecutive D rows x 2 folds):
  pA (bank) = row j  : [0:64] <- T(0,0) fold0 | [64:128] <- T(64,64) fold1
  pB (bank) = row j+1: [0:64] <- T(64,0) fold1 | [64:128] <- T(0,64) fold0 (swapped)
BN is folded into weights/biases on the host. Mask-derived tensors (downsampled
mask, partial-conv rescales) are precomputed on the host from `mask`.
"""
import numpy as np
import ml_dtypes
from contextlib import ExitStack

import concourse.tile as tile
from concourse import bacc, mybir
from concourse.bass_utils import run_bass_kernel_spmd

BF = ml_dtypes.bfloat16
bf16d = mybir.dt.bfloat16
f32d = mybir.dt.float32
AF = mybir.ActivationFunctionType
OP = mybir.AluOpType

B, CIN, D, H, W, C = 4, 32, 64, 64, 64, 64
NJ = 19          # D rows per core region (incl. 3-row halo)
NCORES = 8

# bias column indices in the [128, 16] bias tensor
COL_BD, COL_TD, COL_BF = 0, 1, 14
def COL_B1(i): return 2 + 4 * i
def COL_B2(i): return 3 + 4 * i
def COL_T2(i): return 4 + 4 * i
def COL_B3(i): return 5 + 4 * i


# ----------------------------------------------------------------------------
# host-side preprocessing
# ----------------------------------------------------------------------------

def _bn_fold(p):
    s = (p['g'] / np.sqrt(p['v'] + 1e-5)).astype(np.float64)
    t = (p['b'] - p['m'] * s).astype(np.float64)
    return s, t


def _mask_tensors(mask):
    m = np.asarray(mask, np.float64)[:, 0]                       # [4,64,64,64]
    md = m.reshape(B, 32, 2, 32, 2, 32, 2).max(axis=(2, 4, 6))   # maxpool 2^3
    md = (md > 0.5).astype(np.float64)
    mp = np.pad(md, ((0, 0), (1, 1), (1, 1), (1, 1)))
    cnt3 = np.zeros_like(md)
    for dz in range(3):
        for dy in range(3):
            for dx in range(3):
                cnt3 += mp[:, dz:dz + 32, dy:dy + 32, dx:dx + 32]
    mcm = md * (27.0 / np.maximum(cnt3, 1e-5))
    mp4 = np.pad(m, ((0, 0), (1, 2), (1, 2), (1, 2)))
    cnt4 = np.zeros((B, 32, 32, 32), np.float64)
    for dz in range(4):
        for dy in range(4):
            for dx in range(4):
                cnt4 += mp4[:, dz:dz + 64:2, dy:dy + 64:2, dx:dx + 64:2]
    mc0 = 64.0 / np.maximum(cnt4, 1e-5)
    return md, mcm, mc0


def _pack_weights(params):
    """Fold BN into conv weights/biases; build device layouts."""
    out = {}
    bias = np.zeros((64, 16), np.float64)

    sd, td = _bn_fold(params['down_bn'])
    wd = np.asarray(params['down_w'], np.float64) * sd[:, None, None, None, None]
    # wd2T [128, 32, 64]: row (ci + 32*par [+64*fold dup]), offset o2=dz*8+ky*2+kxp
    # pairs e-grid (kx = 2*kxp+1) with o-grid (kx = 2*kxp)
    wd2 = np.zeros((64, 32, 64), np.float64)   # [ci+32par, o2, co]
    for dz in range(4):
        for ky in range(4):
            for kxp in range(2):
                o2 = dz * 8 + ky * 2 + kxp
                wd2[0:32, o2, :] = wd[:, :, dz, ky, 2 * kxp + 1].T   # par=0 e-grid
                wd2[32:64, o2, :] = wd[:, :, dz, ky, 2 * kxp].T      # par=1 o-grid
    out['wd'] = np.concatenate([wd2, wd2], axis=0).astype(BF)        # [128,32,64]
    bias[:, COL_BD] = sd * np.asarray(params['down_b'], np.float64)
    bias[:, COL_TD] = td

    w1 = np.zeros((64, 3, 64), np.float64)
    w3 = np.zeros((64, 3, 64), np.float64)
    w2 = np.zeros((64, 3, 27, 64), np.float64)
    for i in (1, 2, 3):
        s1, t1 = _bn_fold(params['bn%d_1' % i])
        s2, t2 = _bn_fold(params['bn%d_2' % i])
        s3, t3 = _bn_fold(params['bn%d_3' % i])
        w1[:, i - 1, :] = (np.asarray(params['b%dc1_w' % i], np.float64)[:, :, 0, 0, 0]
                           * s1[:, None]).T
        bias[:, COL_B1(i - 1)] = s1 * np.asarray(params['b%dc1_b' % i], np.float64) + t1
        w2f = np.asarray(params['b%dc2_w' % i], np.float64) * s2[:, None, None, None, None]
        w2[:, i - 1, :, :] = w2f.transpose(1, 2, 3, 4, 0).reshape(64, 27, 64)
        bias[:, COL_B2(i - 1)] = s2 * np.asarray(params['b%dc2_b' % i], np.float64)
        bias[:, COL_T2(i - 1)] = t2
        w3[:, i - 1, :] = (np.asarray(params['b%dc3_w' % i], np.float64)[:, :, 0, 0, 0]
                           * s3[:, None]).T
        bias[:, COL_B3(i - 1)] = s3 * np.asarray(params['b%dc3_b' % i], np.float64) + t3
    sf, tf = _bn_fold(params['fin_bn'])
    wf = (np.asarray(params['fin_w'], np.float64)[:, :, 0, 0, 0] * sf[:, None]).T
    bias[:, COL_BF] = sf * np.asarray(params['fin_b'], np.float64) + tf

    out['w1'] = np.concatenate([w1, w1], axis=0).astype(BF)
    out['w2'] = np.concatenate([w2, w2], axis=0).astype(BF)
    out['w3'] = np.concatenate([w3, w3], axis=0).astype(BF)
    out['wf'] = np.concatenate([wf, wf], axis=0).reshape(128, 64).astype(BF)
    out['bias'] = np.concatenate([bias, bias], axis=0).astype(np.float32)  # [128,16]
    return out


def _pack_core_inputs(feat, md, mcm, mc0, wpack):
    """Per-core input maps. Core c = (b=c//2, g=c%2)."""
    in_maps = []
    for c in range(NCORES):
        b, g = c // 2, c % 2
        fp0 = -1 if g == 0 else 25       # featp plane fp <-> input abs d = fp0+fp
        d0 = 0 if g == 0 else 13         # region row j <-> abs feat1 d = d0 + j-1

        fpad = np.pad(np.asarray(feat[b], np.float32),
                      ((0, 0), (1, 1), (1, 1), (1, 1)))  # idx = abs+1
        # featp2 [128, 40, 34, 33]: p = ci + 32*par + 64*fold
        #   h: input abs h = 32*fold - 1 + hl  (hl 0..33)
        #   w: par0 (e-grid) abs w = 2*wg; par1 (o-grid) abs w = 2*wg - 1
        fsl = fpad[:, fp0 + 1: fp0 + 41]                 # [32, 40, 66, 66]
        featp = np.empty((128, 40, 34, 33), BF)
        for f in range(2):
            hsl = fsl[:, :, 32 * f: 32 * f + 34, :]      # abs h 32f-1 .. 32f+32
            featp[64 * f + 0: 64 * f + 32] = hsl[:, :, :, 1::2]   # abs w 0,2,..,64
            featp[64 * f + 32: 64 * f + 64] = hsl[:, :, :, 0::2]  # abs w -1,1,..,63
        # broadcast site tensors [128, 19, 512]
        def ev(src):
            reg = src[b, d0:d0 + NJ]                     # [19, 32, 32]
            t = np.empty((128, NJ, 512), BF)
            for f in range(2):
                t[64 * f: 64 * f + 64] = np.broadcast_to(
                    reg[:, 16 * f: 16 * f + 16, :].reshape(NJ, 512), (64, NJ, 512))
            return t
        in_maps.append(dict(featp=featp, m_ev=ev(md), mcm_ev=ev(mcm),
                            mc0_ev=ev(mc0), **wpack))
    return in_maps


# ----------------------------------------------------------------------------
# device kernel
# ----------------------------------------------------------------------------

_NC_CACHE = {}


def _build_nc():
    if 'nc' in _NC_CACHE:
        return _NC_CACHE['nc']
    nc = bacc.Bacc("TRN2", target_bir_lowering=False, debug=False,
                   num_devices=NCORES)
    featp_d = nc.dram_tensor("featp", [128, 40, 34, 33], bf16d, kind="ExternalInput").ap()
    m_d = nc.dram_tensor("m_ev", [128, NJ, 512], bf16d, kind="ExternalInput").ap()
    mcm_d = nc.dram_tensor("mcm_ev", [128, NJ, 512], bf16d, kind="ExternalInput").ap()
    mc0_d = nc.dram_tensor("mc0_ev", [128, NJ, 512], bf16d, kind="ExternalInput").ap()
    wd_d = nc.dram_tensor("wd", [128, 32, 64], bf16d, kind="ExternalInput").ap()
    w1_d = nc.dram_tensor("w1", [128, 3, 64], bf16d, kind="ExternalInput").ap()
    w2_d = nc.dram_tensor("w2", [128, 3, 27, 64], bf16d, kind="ExternalInput").ap()
    w3_d = nc.dram_tensor("w3", [128, 3, 64], bf16d, kind="ExternalInput").ap()
    wf_d = nc.dram_tensor("wf", [128, 64], bf16d, kind="ExternalInput").ap()
    bias_d = nc.dram_tensor("bias", [128, 16], f32d, kind="ExternalInput").ap()
    out_d = nc.dram_tensor("out", [128, NJ, 512], f32d, kind="ExternalOutput").ap()

    with tile.TileContext(nc) as tc, ExitStack() as ctx:
        P = ctx.enter_context(tc.tile_pool(name="persist", bufs=1))
        TMP = ctx.enter_context(tc.tile_pool(name="tmp", bufs=1))
        PS = ctx.enter_context(tc.tile_pool(name="psum", bufs=1, space="PSUM"))
        FR = ctx.enter_context(tc.tile_pool(name="fring", bufs=1))

        t_m = P.tile([128, NJ, 512], bf16d)
        nc.sync.dma_start(t_m[:], m_d[:])
        t_mcm = P.tile([128, NJ, 512], bf16d)
        nc.sync.dma_start(t_mcm[:], mcm_d[:])
        t_mc0 = P.tile([128, NJ, 512], bf16d)
        nc.sync.dma_start(t_mc0[:], mc0_d[:])
        t_wd = P.tile([128, 32, 64], bf16d)
        nc.sync.dma_start(t_wd[:], wd_d[:])
        t_w1 = P.tile([128, 3, 64], bf16d)
        nc.sync.dma_start(t_w1[:], w1_d[:])
        t_w2 = P.tile([128, 3, 27, 64], bf16d)
        nc.sync.dma_start(t_w2[:], w2_d[:])
        t_w3 = P.tile([128, 3, 64], bf16d)
        nc.sync.dma_start(t_w3[:], w3_d[:])
        t_wf = P.tile([128, 64], bf16d)
        nc.sync.dma_start(t_wf[:], wf_d[:])
        t_bias = P.tile([128, 16], f32d)
        nc.sync.dma_start(t_bias[:], bias_d[:])

        t_x = P.tile([128, NJ + 2, 512], bf16d)           # j = 0..20 (0,20 absorbers)
        t_f1m = P.tile([128, NJ + 2, 18, 36], bf16d)      # padded c2 input
        t_f2 = P.tile([128, NJ + 2, 512], bf16d)
        nc.gpsimd.memset(t_f1m[:], 0.0)

        # featp ring: tile t covers planes 4t..4t+3
        ftiles = {}

        def ftile(t):
            if t not in ftiles:
                tt = FR.tile([128, 4, 34, 33], bf16d, tag="fp", bufs=3)
                nc.sync.dma_start(tt[:], featp_d[:, 4 * t: 4 * t + 4])
                ftiles[t] = tt
            return ftiles[t]

        def bias_ap(col):
            return t_bias[:, col:col + 1]

        # quad spec: (rowgrp/fold f, col c, psum, j). lhsT/rhs partitions 64f.
        def emit_conv(noff, lhs_fn, rhs_fn, psA, psB, jA):
            specs = [(0, 0, psA, jA), (1, 64, psA, jA)]
            if psB is not None:
                specs += [(1, 0, psB, jA + 1), (0, 64, psB, jA + 1)]
            for o in range(noff):
                for (f, cc, ps, j) in specs:
                    nc.tensor.matmul(
                        ps[cc:cc + 64, :], lhs_fn(o, f), rhs_fn(o, f, j),
                        start=(o == 0), stop=(o == noff - 1),
                        tile_position=(64 * f, cc))

        def f1m_int(j):
            return t_f1m[:, j, 1:17, 2:34]

        def half_bias(col, lo):
            return t_bias[0:64, col:col + 1] if lo else t_bias[64:128, col:col + 1]

        # evacuate a psum bank into an ALIGNED [128,512] sbuf tile, adding a
        # per-channel bias. For swapped banks (pB) the halves are crossed at
        # this step (PSUM-source ops may shift partitions; SBUF-SBUF may not).
        def evac(ps, swapped, bcol, dt=bf16d, lrelu=False, tg="u"):
            u = TMP.tile([128, 512], dt, tag=tg, bufs=4, name="u")
            if not swapped:
                if lrelu:
                    nc.scalar.activation(u[:], ps[:], AF.Lrelu,
                                         bias=bias_ap(bcol), alpha=0.01)
                else:
                    nc.vector.tensor_scalar(u[:], ps[:], bias_ap(bcol), None,
                                            op0=OP.add)
            else:
                if lrelu:
                    nc.scalar.activation(u[0:64, :], ps[64:128, :], AF.Lrelu,
                                         bias=half_bias(bcol, True), alpha=0.01)
                    nc.scalar.activation(u[64:128, :], ps[0:64, :], AF.Lrelu,
                                         bias=half_bias(bcol, False), alpha=0.01)
                else:
                    nc.vector.tensor_scalar(u[0:64, :], ps[64:128, :],
                                            half_bias(bcol, True), None, op0=OP.add)
                    nc.scalar.activation(u[64:128, :], ps[0:64, :], AF.Identity,
                                         bias=half_bias(bcol, False))
            return u

        def epi_c1(blk, ps, j, swapped):
            u = evac(ps, swapped, COL_B1(blk), lrelu=True)
            nc.vector.tensor_tensor(f1m_int(j), u[:], t_m[:, j - 1], op=OP.mult)

        def epi_scaled(ps, j, swapped, bcol, tcol, scale_t, dest):
            # dest[:, j] = lrelu(scale_t[:, j-1] * (ps + bias[bcol]) + bias[tcol])
            u = evac(ps, swapped, bcol)
            v = TMP.tile([128, 512], bf16d, tag="v", bufs=4)
            nc.vector.tensor_tensor(v[:], u[:], scale_t[:, j - 1], op=OP.mult)
            nc.scalar.activation(dest[:, j], v[:], AF.Lrelu,
                                 bias=bias_ap(tcol), alpha=0.01)

        def epi_c3(blk, ps, j, swapped):
            u = evac(ps, swapped, COL_B3(blk))
            q = TMP.tile([128, 512], bf16d, tag="q", bufs=4)
            nc.vector.tensor_tensor(q[:], u[:], t_x[:, j], op=OP.add)
            nc.scalar.activation(t_x[:, j], q[:], AF.Lrelu, alpha=0.01)

        def epi_fin(ps, j, swapped):
            u = evac(ps, swapped, COL_BF, dt=f32d, tg="uf")
            st = TMP.tile([128, 512], f32d, tag="st", bufs=2)
            nc.vector.tensor_tensor(st[:], u[:], t_m[:, j - 1], op=OP.mult)
            nc.sync.dma_start(out_d[:, j - 1], st[:])

        def quad_rows():
            for jA in range(1, NJ + 1, 2):
                yield jA, (jA + 1 <= NJ)

        # ------------------- down conv -------------------
        def dn_lhs(o, f):
            return t_wd[64 * f: 64 * f + 64, o, :]

        def dn_rhs(o, f, j):
            dz, r = divmod(o, 8)
            ky, kxp = divmod(r, 2)
            fp = 2 * j - 2 + dz
            ft = ftile(fp // 4)
            return ft[64 * f: 64 * f + 64, fp % 4, ky:ky + 31:2, kxp:kxp + 32]

        for jA, full in quad_rows():
            pA = PS.tile([128, 512], f32d, tag="pA", bufs=2, name="pA")
            pB = PS.tile([128, 512], f32d, tag="pB", bufs=2, name="pB") if full else None
            emit_conv(32, dn_lhs, dn_rhs, pA, pB, jA)
            epi_scaled(pA, jA, False, COL_BD, COL_TD, t_mc0, t_x)
            if full:
                epi_scaled(pB, jA + 1, True, COL_BD, COL_TD, t_mc0, t_x)

        # ------------------- res blocks -------------------
        for blk in range(3):
            # c1 + mask
            def c1_lhs(o, f, _b=blk):
                return t_w1[64 * f: 64 * f + 64, _b, :]

            def c1_rhs(o, f, j):
                return t_x[64 * f: 64 * f + 64, j]

            for jA, full in quad_rows():
                pA = PS.tile([128, 512], f32d, tag="pA", bufs=2, name="pA")
                pB = PS.tile([128, 512], f32d, tag="pB", bufs=2, name="pB") if full else None
                emit_conv(1, c1_lhs, c1_rhs, pA, pB, jA)
                epi_c1(blk, pA, jA, False)
                if full:
                    epi_c1(blk, pB, jA + 1, True)

            # f1m halo rows (cross-fold)
            nc.gpsimd.tensor_copy(t_f1m[0:64, 1:NJ + 1, 17:18, 2:34],
                                  t_f1m[64:128, 1:NJ + 1, 1:2, 2:34])
            nc.gpsimd.tensor_copy(t_f1m[64:128, 1:NJ + 1, 0:1, 2:34],
                                  t_f1m[0:64, 1:NJ + 1, 16:17, 2:34])

            # c2 (3x3x3) + mc
            def c2_lhs(o, f, _b=blk):
                return t_w2[64 * f: 64 * f + 64, _b, o, :]

            def c2_rhs(o, f, j):
                dz, r = divmod(o, 9)
                dy, dx = divmod(r, 3)
                return t_f1m[64 * f: 64 * f + 64, j + dz - 1,
                             dy:dy + 16, 1 + dx:33 + dx]

            for jA, full in quad_rows():
                pA = PS.tile([128, 512], f32d, tag="pA", bufs=2, name="pA")
                pB = PS.tile([128, 512], f32d, tag="pB", bufs=2, name="pB") if full else None
                emit_conv(27, c2_lhs, c2_rhs, pA, pB, jA)
                epi_scaled(pA, jA, False, COL_B2(blk), COL_T2(blk), t_mcm, t_f2)
                if full:
                    epi_scaled(pB, jA + 1, True, COL_B2(blk), COL_T2(blk), t_mcm, t_f2)

            # c3 + residual
            def c3_lhs(o, f, _b=blk):
                return t_w3[64 * f: 64 * f + 64, _b, :]

            def c3_rhs(o, f, j):
                return t_f2[64 * f: 64 * f + 64, j]

            for jA, full in quad_rows():
                pA = PS.tile([128, 512], f32d, tag="pA", bufs=2, name="pA")
                pB = PS.tile([128, 512], f32d, tag="pB", bufs=2, name="pB") if full else None
                emit_conv(1, c3_lhs, c3_rhs, pA, pB, jA)
                epi_c3(blk, pA, jA, False)
                if full:
                    epi_c3(blk, pB, jA + 1, True)

        # ------------------- final conv -------------------
        def fin_lhs(o, f):
            return t_wf[64 * f: 64 * f + 64, :]

        def fin_rhs(o, f, j):
            return t_x[64 * f: 64 * f + 64, j]

        for jA, full in quad_rows():
            pA = PS.tile([128, 512], f32d, tag="pA", bufs=2, name="pA")
            pB = PS.tile([128, 512], f32d, tag="pB", bufs=2, name="pB") if full else None
            emit_conv(1, fin_lhs, fin_rhs, pA, pB, jA)
            epi_fin(pA, jA, False)
            if full:
                epi_fin(pB, jA + 1, True)

    nc.compile()
    _NC_CACHE['nc'] = nc
    return nc


# ----------------------------------------------------------------------------
# entry point
# ----------------------------------------------------------------------------

def kernel(feat, mask, params):
    feat = np.asarray(feat)
    mask = np.asarray(mask)
    md, mcm, mc0 = _mask_tensors(mask)
    wpack = _pack_weights(params)
    in_maps = _pack_core_inputs(feat, md, mcm, mc0, wpack)
    nc = _build_nc()
    res = run_bass_kernel_spmd(nc, in_maps, list(range(NCORES))).results

    out = np.empty((B, C, 32, 32, 32), np.float32)
    for c in range(NCORES):
        b, g = c // 2, c % 2
        o = res[c]["out"].reshape(128, NJ, 16, 32)
        j0 = 0 if g == 0 else 3          # row j=1..16 (idx 0..15) / j=4..19 (idx 3..18)
        for f in range(2):
            out[b, :, 16 * g:16 * g + 16, 16 * f:16 * f + 16, :] = \
                o[64 * f:64 * f + 64, j0:j0 + 16]
    mask_out = md[:, None].astype(np.float32)
    return out, mask_out


# revision 6
# speedup vs baseline: 634.0331x; 634.0331x over previous
"""Trainium2 Bass kernel for CarefulSparseResNet3D.

Sharding: 8 cores = batch(4) x D-half(2). Each core computes its output
D-half from an overlapping input slice (compute-with-halo, no collectives).

Device layout: activations are H-fold partitioned: partition p = ch + 64*fold,
fold = H half (h 0..15 / 16..31). All convs run as 4-way PE-tiled (64x64)
matmul accumulations over "quads" (2 consecutive D rows x 2 folds):
  pA (bank) = row j  : [0:64] <- T(0,0) fold0 | [64:128] <- T(64,64) fold1
  pB (bank) = row j+1: [0:64] <- T(64,0) fold1 | [64:128] <- T(0,64) fold0 (swapped)
BN is folded into weights/biases on the host. Mask-derived tensors (downsampled
mask, partial-conv rescales) are precomputed on the host from `mask`.
"""
import numpy as np
import ml_dtypes
from contextlib import ExitStack

import concourse.tile as tile
from concourse import bacc, mybir
from concourse.bass_utils import run_bass_kernel_spmd

BF = ml_dtypes.bfloat16
bf16d = mybir.dt.bfloat16
f32d = mybir.dt.float32
AF = mybir.ActivationFunctionType
OP = mybir.AluOpType

B, CIN, D, H, W, C = 4, 32, 64, 64, 64, 64
NJ = 19          # D rows per core region (incl. 3-row halo)
NCORES = 8

# bias column indices in the [128, 16] bias tensor
COL_BD, COL_TD, COL_BF = 0, 1, 14
def COL_B1(i): return 2 + 4 * i
def COL_B2(i): return 3 + 4 * i
def COL_T2(i): return 4 + 4 * i
def COL_B3(i): return 5 + 4 * i


# ----------------------------------------------------------------------------
# host-side preprocessing
# ----------------------------------------------------------------------------

def _bn_fold(p):
    s = (p['g'] / np.sqrt(p['v'] + 1e-5)).astype(np.float64)
    t = (p['b'] - p['m'] * s).astype(np.float64)
    return s, t


def _mask_tensors(mask):
    m = np.asarray(mask, np.float64)[:, 0]                       # [4,64,64,64]
    md = m.reshape(B, 32, 2, 32, 2, 32, 2).max(axis=(2, 4, 6))   # maxpool 2^3
    md = (md > 0.5).astype(np.float64)
    mp = np.pad(md, ((0, 0), (1, 1), (1, 1), (1, 1)))
    cnt3 = np.zeros_like(md)
    for dz in range(3):
        for dy in range(3):
            for dx in range(3):
                cnt3 += mp[:, dz:dz + 32, dy:dy + 32, dx:dx + 32]
    mcm = md * (27.0 / np.maximum(cnt3, 1e-5))
    mp4 = np.pad(m, ((0, 0), (1, 2), (1, 2), (1, 2)))
    cnt4 = np.zeros((B, 32, 32, 32), np.float64)
    for dz in range(4):
        for dy in range(4):
            for dx in range(4):
                cnt4 += mp4[:, dz:dz + 64:2, dy:dy + 64:2, dx:dx + 64:2]
    mc0 = 64.0 / np.maximum(cnt4, 1e-5)
    return md, mcm, mc0


def _pack_weights(params):
    """Fold BN into conv weights/biases; build device layouts."""
    out = {}
    bias = np.zeros((64, 16), np.float64)

    sd, td = _bn_fold(params['down_bn'])
    wd = np.asarray(params['down_w'], np.float64) * sd[:, None, None, None, None]
    # wd2T [128, 32, 64]: row (ci + 32*par [+64*fold dup]), offset o2=dz*8+ky*2+kxp
    # pairs e-grid (kx = 2*kxp+1) with o-grid (kx = 2*kxp)
    wd2 = np.zeros((64, 32, 64), np.float64)   # [ci+32par, o2, co]
    for dz in range(4):
        for ky in range(4):
            for kxp in range(2):
                o2 = dz * 8 + ky * 2 + kxp
                wd2[0:32, o2, :] = wd[:, :, dz, ky, 2 * kxp + 1].T   # par=0 e-grid
                wd2[32:64, o2, :] = wd[:, :, dz, ky, 2 * kxp].T      # par=1 o-grid
    out['wd'] = np.concatenate([wd2, wd2], axis=0).astype(BF)        # [128,32,64]
    bias[:, COL_BD] = sd * np.asarray(params['down_b'], np.float64)
    bias[:, COL_TD] = td

    w1 = np.zeros((64, 3, 64), np.float64)
    w3 = np.zeros((64, 3, 64), np.float64)
    w2 = np.zeros((64, 3, 27, 64), np.float64)
    for i in (1, 2, 3):
        s1, t1 = _bn_fold(params['bn%d_1' % i])
        s2, t2 = _bn_fold(params['bn%d_2' % i])
        s3, t3 = _bn_fold(params['bn%d_3' % i])
        w1[:, i - 1, :] = (np.asarray(params['b%dc1_w' % i], np.float64)[:, :, 0, 0, 0]
                           * s1[:, None]).T
        bias[:, COL_B1(i - 1)] = s1 * np.asarray(params['b%dc1_b' % i], np.float64) + t1
        w2f = np.asarray(params['b%dc2_w' % i], np.float64) * s2[:, None, None, None, None]
        w2[:, i - 1, :, :] = w2f.transpose(1, 2, 3, 4, 0).reshape(64, 27, 64)
        bias[:, COL_B2(i - 1)] = s2 * np.asarray(params['b%dc2_b' % i], np.float64)
        bias[:, COL_T2(i - 1)] = t2
        w3[:, i - 1, :] = (np.asarray(params['b%dc3_w' % i], np.float64)[:, :, 0, 0, 0]
                           * s3[:, None]).T
        bias[:, COL_B3(i - 1)] = s3 * np.asarray(params['b%dc3_b' % i], np.float64) + t3
    sf, tf = _bn_fold(params['fin_bn'])
    wf = (np.asarray(params['fin_w'], np.float64)[:, :, 0, 0, 0] * sf[:, None]).T
    bias[:, COL_BF] = sf * np.asarray(params['fin_b'], np.float64) + tf

    out['w1'] = np.concatenate([w1, w1], axis=0).astype(BF)
    out['w2'] = np.concatenate([w2, w2], axis=0).astype(BF)
    out['w3'] = np.concatenate([w3, w3], axis=0).astype(BF)
    out['wf'] = np.concatenate([wf, wf], axis=0).reshape(128, 64).astype(BF)
    out['bias'] = np.concatenate([bias, bias], axis=0).astype(np.float32)  # [128,16]
    return out


def _pack_core_inputs(feat, md, mcm, mc0, wpack):
    """Per-core input maps. Core c = (b=c//2, g=c%2)."""
    in_maps = []
    for c in range(NCORES):
        b, g = c // 2, c % 2
        fp0 = -1 if g == 0 else 25       # featp plane fp <-> input abs d = fp0+fp
        d0 = 0 if g == 0 else 13         # region row j <-> abs feat1 d = d0 + j-1

        fpad = np.pad(np.asarray(feat[b], np.float32),
                      ((0, 0), (1, 1), (1, 1), (1, 1)))  # idx = abs+1
        # featp2 [128, 40, 34, 33]: p = ci + 32*par + 64*fold
        #   h: input abs h = 32*fold - 1 + hl  (hl 0..33)
        #   w: par0 (e-grid) abs w = 2*wg; par1 (o-grid) abs w = 2*wg - 1
        fsl = fpad[:, fp0 + 1: fp0 + 41]                 # [32, 40, 66, 66]
        featp = np.empty((128, 40, 34, 33), BF)
        for f in range(2):
            hsl = fsl[:, :, 32 * f: 32 * f + 34, :]      # abs h 32f-1 .. 32f+32
            featp[64 * f + 0: 64 * f + 32] = hsl[:, :, :, 1::2]   # abs w 0,2,..,64
            featp[64 * f + 32: 64 * f + 64] = hsl[:, :, :, 0::2]  # abs w -1,1,..,63
        # broadcast site tensors [128, 19, 512]
        def ev(src):
            reg = src[b, d0:d0 + NJ]                     # [19, 32, 32]
            t = np.empty((128, NJ, 512), BF)
            for f in range(2):
                t[64 * f: 64 * f + 64] = np.broadcast_to(
                    reg[:, 16 * f: 16 * f + 16, :].reshape(NJ, 512), (64, NJ, 512))
            return t
        in_maps.append(dict(featp=featp, m_ev=ev(md), mcm_ev=ev(mcm),
                            mc0_ev=ev(mc0), **wpack))
    return in_maps


# ----------------------------------------------------------------------------
# device kernel
# ----------------------------------------------------------------------------

_NC_CACHE = {}


def _build_nc(reps=1):
    if reps in _NC_CACHE:
        return _NC_CACHE[reps]
    nc = bacc.Bacc("TRN2", target_bir_lowering=False, debug=False,
                   num_devices=NCORES)
    featp_d = nc.dram_tensor("featp", [128, 40, 34, 33], bf16d, kind="ExternalInput").ap()
    m_d = nc.dram_tensor("m_ev", [128, NJ, 512], bf16d, kind="ExternalInput").ap()
    mcm_d = nc.dram_tensor("mcm_ev", [128, NJ, 512], bf16d, kind="ExternalInput").ap()
    mc0_d = nc.dram_tensor("mc0_ev", [128, NJ, 512], bf16d, kind="ExternalInput").ap()
    wd_d = nc.dram_tensor("wd", [128, 32, 64], bf16d, kind="ExternalInput").ap()
    w1_d = nc.dram_tensor("w1", [128, 3, 64], bf16d, kind="ExternalInput").ap()
    w2_d = nc.dram_tensor("w2", [128, 3, 27, 64], bf16d, kind="ExternalInput").ap()
    w3_d = nc.dram_tensor("w3", [128, 3, 64], bf16d, kind="ExternalInput").ap()
    wf_d = nc.dram_tensor("wf", [128, 64], bf16d, kind="ExternalInput").ap()
    bias_d = nc.dram_tensor("bias", [128, 16], f32d, kind="ExternalInput").ap()
    out_d = nc.dram_tensor("out", [128, NJ, 512], f32d, kind="ExternalOutput").ap()

    with tile.TileContext(nc) as tc, ExitStack() as ctx:
        P = ctx.enter_context(tc.tile_pool(name="persist", bufs=1))
        TMP = ctx.enter_context(tc.tile_pool(name="tmp", bufs=1))
        PS = ctx.enter_context(tc.tile_pool(name="psum", bufs=1, space="PSUM"))
        FR = ctx.enter_context(tc.tile_pool(name="fring", bufs=1))

        t_m = P.tile([128, NJ, 512], bf16d)
        nc.sync.dma_start(t_m[:], m_d[:])
        t_mcm = P.tile([128, NJ, 512], bf16d)
        nc.sync.dma_start(t_mcm[:], mcm_d[:])
        t_mc0 = P.tile([128, NJ, 512], bf16d)
        nc.sync.dma_start(t_mc0[:], mc0_d[:])
        t_wd = P.tile([128, 32, 64], bf16d)
        nc.sync.dma_start(t_wd[:], wd_d[:])
        t_w1 = P.tile([128, 3, 64], bf16d)
        nc.sync.dma_start(t_w1[:], w1_d[:])
        t_w2 = P.tile([128, 3, 27, 64], bf16d)
        nc.sync.dma_start(t_w2[:], w2_d[:])
        t_w3 = P.tile([128, 3, 64], bf16d)
        nc.sync.dma_start(t_w3[:], w3_d[:])
        t_wf = P.tile([128, 64], bf16d)
        nc.sync.dma_start(t_wf[:], wf_d[:])
        t_bias = P.tile([128, 16], f32d)
        nc.sync.dma_start(t_bias[:], bias_d[:])

        t_x = P.tile([128, NJ + 2, 512], bf16d)           # j = 0..20 (0,20 absorbers)
        t_f1m = P.tile([128, NJ + 2, 18, 36], bf16d)      # padded c2 input
        t_f2 = P.tile([128, NJ + 2, 512], bf16d)
        nc.gpsimd.memset(t_f1m[:], 0.0)

        loop_ctx = tc.For_i(0, reps, 1) if reps > 1 else None
        if loop_ctx is not None:
            loop_ctx.__enter__()

        # featp ring: tile t covers planes 4t..4t+3
        ftiles = {}

        def ftile(t):
            if t not in ftiles:
                tt = FR.tile([128, 4, 34, 33], bf16d, tag="fp", bufs=3)
                nc.sync.dma_start(tt[:], featp_d[:, 4 * t: 4 * t + 4])
                ftiles[t] = tt
            return ftiles[t]

        def bias_ap(col):
            return t_bias[:, col:col + 1]

        # quad spec: (rowgrp/fold f, col c, psum, j). lhsT/rhs partitions 64f.
        def emit_conv(noff, lhs_fn, rhs_fn, psA, psB, jA):
            specs = [(0, 0, psA, jA), (1, 64, psA, jA)]
            if psB is not None:
                specs += [(1, 0, psB, jA + 1), (0, 64, psB, jA + 1)]
            for o in range(noff):
                for (f, cc, ps, j) in specs:
                    nc.tensor.matmul(
                        ps[cc:cc + 64, :], lhs_fn(o, f), rhs_fn(o, f, j),
                        start=(o == 0), stop=(o == noff - 1),
                        tile_position=(64 * f, cc))

        def f1m_int(j):
            return t_f1m[:, j, 1:17, 2:34]

        def half_bias(col, lo):
            return t_bias[0:64, col:col + 1] if lo else t_bias[64:128, col:col + 1]

        # evacuate a psum bank into an ALIGNED [128,512] sbuf tile, adding a
        # per-channel bias. For swapped banks (pB) the halves are crossed at
        # this step (PSUM-source ops may shift partitions; SBUF-SBUF may not).
        def evac(ps, swapped, bcol, dt=bf16d, lrelu=False, tg="u"):
            u = TMP.tile([128, 512], dt, tag=tg, bufs=4, name="u")
            if not swapped:
                if lrelu:
                    nc.scalar.activation(u[:], ps[:], AF.Lrelu,
                                         bias=bias_ap(bcol), alpha=0.01)
                else:
                    nc.vector.tensor_scalar(u[:], ps[:], bias_ap(bcol), None,
                                            op0=OP.add)
            else:
                if lrelu:
                    nc.scalar.activation(u[0:64, :], ps[64:128, :], AF.Lrelu,
                                         bias=half_bias(bcol, True), alpha=0.01)
                    nc.scalar.activation(u[64:128, :], ps[0:64, :], AF.Lrelu,
                                         bias=half_bias(bcol, False), alpha=0.01)
                else:
                    nc.vector.tensor_scalar(u[0:64, :], ps[64:128, :],
                                            half_bias(bcol, True), None, op0=OP.add)
                    nc.scalar.activation(u[64:128, :], ps[0:64, :], AF.Identity,
                                         bias=half_bias(bcol, False))
            return u

        def epi_c1(blk, ps, j, swapped):
            u = evac(ps, swapped, COL_B1(blk), lrelu=True)
            nc.vector.tensor_tensor(f1m_int(j), u[:], t_m[:, j - 1], op=OP.mult)

        def epi_scaled(ps, j, swapped, bcol, tcol, scale_t, dest):
            # dest[:, j] = lrelu(scale_t[:, j-1] * (ps + bias[bcol]) + bias[tcol])
            u = evac(ps, swapped, bcol)
            v = TMP.tile([128, 512], bf16d, tag="v", bufs=4)
            nc.vector.tensor_tensor(v[:], u[:], scale_t[:, j - 1], op=OP.mult)
            nc.scalar.activation(dest[:, j], v[:], AF.Lrelu,
                                 bias=bias_ap(tcol), alpha=0.01)

        def epi_c3(blk, ps, j, swapped):
            u = evac(ps, swapped, COL_B3(blk))
            q = TMP.tile([128, 512], bf16d, tag="q", bufs=4)
            nc.vector.tensor_tensor(q[:], u[:], t_x[:, j], op=OP.add)
            nc.scalar.activation(t_x[:, j], q[:], AF.Lrelu, alpha=0.01)

        def epi_fin(ps, j, swapped):
            u = evac(ps, swapped, COL_BF, dt=f32d, tg="uf")
            st = TMP.tile([128, 512], f32d, tag="st", bufs=2)
            nc.vector.tensor_tensor(st[:], u[:], t_m[:, j - 1], op=OP.mult)
            nc.sync.dma_start(out_d[:, j - 1], st[:])

        def quad_rows():
            for jA in range(1, NJ + 1, 2):
                yield jA, (jA + 1 <= NJ)

        # ------------------- down conv -------------------
        def dn_lhs(o, f):
            return t_wd[64 * f: 64 * f + 64, o, :]

        def dn_rhs(o, f, j):
            dz, r = divmod(o, 8)
            ky, kxp = divmod(r, 2)
            fp = 2 * j - 2 + dz
            ft = ftile(fp // 4)
            return ft[64 * f: 64 * f + 64, fp % 4, ky:ky + 31:2, kxp:kxp + 32]

        for jA, full in quad_rows():
            pA = PS.tile([128, 512], f32d, tag="pA", bufs=2, name="pA")
            pB = PS.tile([128, 512], f32d, tag="pB", bufs=2, name="pB") if full else None
            emit_conv(32, dn_lhs, dn_rhs, pA, pB, jA)
            epi_scaled(pA, jA, False, COL_BD, COL_TD, t_mc0, t_x)
            if full:
                epi_scaled(pB, jA + 1, True, COL_BD, COL_TD, t_mc0, t_x)

        # ------------------- res blocks -------------------
        for blk in range(3):
            # c1 + mask
            def c1_lhs(o, f, _b=blk):
                return t_w1[64 * f: 64 * f + 64, _b, :]

            def c1_rhs(o, f, j):
                return t_x[64 * f: 64 * f + 64, j]

            for jA, full in quad_rows():
                pA = PS.tile([128, 512], f32d, tag="pA", bufs=2, name="pA")
                pB = PS.tile([128, 512], f32d, tag="pB", bufs=2, name="pB") if full else None
                emit_conv(1, c1_lhs, c1_rhs, pA, pB, jA)
                epi_c1(blk, pA, jA, False)
                if full:
                    epi_c1(blk, pB, jA + 1, True)

            # f1m halo rows (cross-fold)
            nc.gpsimd.tensor_copy(t_f1m[0:64, 1:NJ + 1, 17:18, 2:34],
                                  t_f1m[64:128, 1:NJ + 1, 1:2, 2:34])
            nc.gpsimd.tensor_copy(t_f1m[64:128, 1:NJ + 1, 0:1, 2:34],
                                  t_f1m[0:64, 1:NJ + 1, 16:17, 2:34])

            # c2 (3x3x3) + mc
            def c2_lhs(o, f, _b=blk):
                return t_w2[64 * f: 64 * f + 64, _b, o, :]

            def c2_rhs(o, f, j):
                dz, r = divmod(o, 9)
                dy, dx = divmod(r, 3)
                return t_f1m[64 * f: 64 * f + 64, j + dz - 1,
                             dy:dy + 16, 1 + dx:33 + dx]

            for jA, full in quad_rows():
                pA = PS.tile([128, 512], f32d, tag="pA", bufs=2, name="pA")
                pB = PS.tile([128, 512], f32d, tag="pB", bufs=2, name="pB") if full else None
                emit_conv(27, c2_lhs, c2_rhs, pA, pB, jA)
                epi_scaled(pA, jA, False, COL_B2(blk), COL_T2(blk), t_mcm, t_f2)
                if full:
                    epi_scaled(pB, jA + 1, True, COL_B2(blk), COL_T2(blk), t_mcm, t_f2)

            # c3 + residual
            def c3_lhs(o, f, _b=blk):
                return t_w3[64 * f: 64 * f + 64, _b, :]

            def c3_rhs(o, f, j):
                return t_f2[64 * f: 64 * f + 64, j]

            for jA, full in quad_rows():
                pA = PS.tile([128, 512], f32d, tag="pA", bufs=2, name="pA")
                pB = PS.tile([128, 512], f32d, tag="pB", bufs=2, name="pB") if full else None
                emit_conv(1, c3_lhs, c3_rhs, pA, pB, jA)
                epi_c3(blk, pA, jA, False)
                if full:
                    epi_c3(blk, pB, jA + 1, True)

        # ------------------- final conv -------------------
        def fin_lhs(o, f):
            return t_wf[64 * f: 64 * f + 64, :]

        def fin_rhs(o, f, j):
            return t_x[64 * f: 64 * f + 64, j]

        for jA, full in quad_rows():
            pA = PS.tile([128, 512], f32d, tag="pA", bufs=2, name="pA")
            pB = PS.tile([128, 512], f32d, tag="pB", bufs=2, name="pB") if full else None
            emit_conv(1, fin_lhs, fin_rhs, pA, pB, jA)
            epi_fin(pA, jA, False)
            if full:
                epi_fin(pB, jA + 1, True)

        if loop_ctx is not None:
            loop_ctx.__exit__(None, None, None)

    nc.compile()
    _NC_CACHE[reps] = nc
    return nc


# ----------------------------------------------------------------------------
# entry point
# ----------------------------------------------------------------------------

def kernel(feat, mask, params):
    feat = np.asarray(feat)
    mask = np.asarray(mask)
    md, mcm, mc0 = _mask_tensors(mask)
    wpack = _pack_weights(params)
    in_maps = _pack_core_inputs(feat, md, mcm, mc0, wpack)
    nc = _build_nc()
    res = run_bass_kernel_spmd(nc, in_maps, list(range(NCORES))).results

    out = np.empty((B, C, 32, 32, 32), np.float32)
    for c in range(NCORES):
        b, g = c // 2, c % 2
        o = res[c]["out"].reshape(128, NJ, 16, 32)
        j0 = 0 if g == 0 else 3          # row j=1..16 (idx 0..15) / j=4..19 (idx 3..18)
        for f in range(2):
            out[b, :, 16 * g:16 * g + 16, 16 * f:16 * f + 16, :] = \
                o[64 * f:64 * f + 64, j0:j0 + 16]
    mask_out = md[:, None].astype(np.float32)
    return out, mask_out
